# revision 25
# baseline (speedup 1.0000x reference)
"""Attention-pooling kernel for TRN2 (8 NeuronCores, data-parallel over batch).

Computes, per batch b:
    scores = seeds @ x[b].T          # [M, S]
    weights = softmax(scores, -1)
    out[b] = weights @ x[b]          # [M, D]

Sharding: batch B=32 split 4-per-core across 8 cores; seeds replicated.

Per-core pipeline (all bf16 on-chip, f32 PSUM accumulation):
  - SWDGE cast-DMA loads x HBM f32 -> SBUF bf16 in tapered chunks
    (1-4 MB: small at the very start to fill the pipeline early and at
    the very end to shrink the post-stream compute tail; 4 MB in the
    middle where only line rate matters). The x stream is the critical
    path (~67 MB at ~337 GB/s effective HBM rate = ~199 us), so the
    chunk DMAs are the first instructions on the gpsimd queue and the
    seeds load rides the HWDGE (scalar) queue instead.
  - PE transposes x 128x128 blocks (bf16) -> psum -> DVE copies to SBUF
    as x^T chunks.
  - scores: 4 accumulating matmuls lhsT=seedsT chunk [128,16], rhs=xT.
  - exp on ACT straight out of PSUM, with fused accum_out row-sums
    (no max subtraction: scores = seeds.x are bounded ~|8|, exp is safe
    in f32).
  - PE transposes exp [16,128] -> expT [128,16]; pooled matmuls are
    4-way column-tiled (concurrent PE col-groups), partials accumulate
    in psum[32q:32q+16, :] over the whole batch.
  - batch end: reduce partials + recip(sum) on DVE, scale, DMA out f32.
  - Stages are software-pipelined (C(i-2), B(i-1), A(i)) so PE never
    waits on the ACT/DVE round trip of the same macro-tile.
"""

from contextlib import ExitStack

import numpy as np

import concourse.mybir as mybir
import concourse.tile as tile
from concourse import bacc
from concourse.bass_utils import run_bass_kernel_spmd
from concourse.masks import make_identity

N_CORES = 8
B, S, D, M = 32, 8192, 512, 16
S_MACRO = 512           # s rows per macro-tile
T_SUB = S_MACRO // 128  # 128-row subtiles per macro-tile
DC = D // 128            # 128-col d chunks
XP_BUFS = 16             # 1 MB chunk tiles in flight (4 KB/partition each)

f32 = mybir.dt.float32
bf16 = mybir.dt.bfloat16


def batch_macros(bb, b_loc, n_q):
    """(q0, nq) macro extents for one batch, in 128-row q units.

    One DMA chunk == one macro. Uniform 4-q (1 MB) macros: the SWDGE
    stream pipelines packets continuously regardless of chunk size, and
    per-macro completion granularity keeps the PE's data waits tiny
    (~0.3 us) so the PE_HAM activity monitor never sees an idle window
    and never half-clocks the PE (4 MB chunks caused ~2 us waits -> K=4
    windows -> ~2.5 us of extra PE time after every chunk boundary).
    The very last two macros are split into four 2-q (0.5 MB) minis so
    the post-stream dependency chain (transpose -> copy -> scores ->
    exp -> eT -> pooled) on the final macro is half as long.
    """
    if bb < b_loc - 1:
        return [(4 * k, 4) for k in range(n_q // 4)]
    n4 = n_q // 4 - 2
    return [(4 * k, 4) for k in range(n4)] + \
           [(4 * n4 + 2 * k, 2) for k in range(4)]


def kernel_body(tc, out_ap, x_ap, seeds_ap, b_loc, s):
    nc = tc.nc
    n_mac = s // S_MACRO
    with ExitStack() as ctx:
        const = ctx.enter_context(tc.tile_pool(name="const", bufs=1))
        xp = ctx.enter_context(tc.tile_pool(name="xp", bufs=XP_BUFS))
        xtp = ctx.enter_context(tc.tile_pool(name="xtp", bufs=4))
        ep = ctx.enter_context(tc.tile_pool(name="ep", bufs=4))
        etp = ctx.enter_context(tc.tile_pool(name="etp", bufs=4))
        statp = ctx.enter_context(tc.tile_pool(name="statp", bufs=4))
        outp = ctx.enter_context(tc.tile_pool(name="outp", bufs=2))
        ps_xt = ctx.enter_context(tc.tile_pool(name="ps_xt", bufs=4, space="PSUM"))
        # sc bufs=2: scores(i) must not wait for exp(i-1) to drain the
        # scores psum — that PE<->ACT ping-pong serializes the post-stream
        # drain. pl bufs=1 is safe: the next batch's first pooled matmul
        # lands ~2.6 us after the previous batch's finalize reads.
        ps_sc = ctx.enter_context(tc.tile_pool(name="ps_sc", bufs=2, space="PSUM"))
        ps_et = ctx.enter_context(tc.tile_pool(name="ps_et", bufs=1, space="PSUM"))
        ps_pl = ctx.enter_context(tc.tile_pool(name="ps_pl", bufs=1, space="PSUM"))

        # x view per batch: [b, p, q, d] with s = p*(s/128) + q. Partition p
        # holds s/128=64 consecutive s rows (128 KB contiguous HBM), so any
        # q-slice is a contiguous per-partition read. The s-order is a fixed
        # permutation; softmax is permutation-invariant and scores/exp/pooled
        # all use the same block mapping, so it cancels.
        x_rb = x_ap.rearrange("b (p q) d -> b p q d", p=128)

        # macro schedule: (bb, j within batch, q0, nq); chunk == macro
        n_q = s // 128
        macros = []
        n_eff = {}
        for bb in range(b_loc):
            bm = batch_macros(bb, b_loc, n_q)
            n_eff[bb] = len(bm)
            for j, (q0, nq) in enumerate(bm):
                macros.append((bb, j, q0, nq))
        NM = len(macros)

        chunk_tiles = {}

        def stage_dma(i):
            bb, j, q0, nq = macros[i]
            x_bf = xp.tile([128, T_SUB, D], bf16, tag="x", name="x_bf")
            nc.gpsimd.dma_start(
                out=x_bf[:, :nq, :],
                in_=x_rb[bb, :, q0:q0 + nq, :],
            )
            chunk_tiles[i] = x_bf

        # x chunk DMAs are the first gpsimd-queue work so SDMA starts
        # streaming immediately; identity (also gpsimd: memset +
        # affine_select) slots in after two emissions, well before the
        # first transposes need it.
        PREFETCH = XP_BUFS - 1
        stage_dma(0)
        stage_dma(1)

        ident = const.tile([128, 128], bf16)
        make_identity(nc, ident)

        # seeds on the HWDGE (scalar) queue, f32; DVE casts to bf16.
        seeds_f = const.tile([M, D], f32)
        nc.scalar.dma_start(out=seeds_f[:], in_=seeds_ap)
        seeds_bf = const.tile([M, D], bf16)
        nc.vector.tensor_copy(seeds_bf[:], seeds_f[:])

        for c in range(2, min(PREFETCH, NM)):
            stage_dma(c)

        # seeds -> seedsT [d, m] chunks, [128, DC*M] (dc-major)
        ps_st = ps_et.tile([128, DC * M], bf16, tag="et", name="et")
        for dc in range(DC):
            nc.tensor.transpose(
                ps_st[:, dc * M:(dc + 1) * M],
                seeds_bf[:, dc * 128:(dc + 1) * 128],
                ident[:M, :M],
            )
        seedsT = const.tile([128, DC * M], bf16)
        nc.vector.tensor_copy(seedsT[:], ps_st[:])

        # Software-pipelined across all (batch, macro) pairs:
        #   DMA: one cast-load per chunk, PREFETCH chunks ahead
        #   stage A(i): PE x-transposes, DVE psum->sbuf copies
        #   stage B(i): scores matmuls, ACT exp (+row-sum)
        #   stage C(i): PE exp-transposes, DVE copy, pooled matmuls, finalize
        # B runs 2 macros behind A and C 3 behind, so every cross-engine
        # round trip (PE->DVE xt copy->scores; ACT exp->eT) has a full
        # macro of slack and the PE never stalls mid-macro.
        st = {}  # per-macro live tiles
        batch = {}  # per-batch state: sums tile, pool psum

        def stage_a(i):
            bb, j, q0, nq = macros[i]
            x_bf = chunk_tiles[i]
            w = nq * 128
            xt_sb = xtp.tile([128, DC, S_MACRO], bf16, tag="xt", name="xt")
            for ph in range(DC // 2):  # 2 dc chunks per psum bank
                xt_ps = ps_xt.tile([128, 2 * S_MACRO], bf16, tag="xt", name="xt")
                for dch in range(2):
                    dc = ph * 2 + dch
                    for t in range(nq):
                        nc.tensor.transpose(
                            xt_ps[:, dch * w + t * 128:dch * w + (t + 1) * 128],
                            x_bf[:, t, dc * 128:(dc + 1) * 128],
                            ident[:],
                        )
                if nq == T_SUB:
                    nc.vector.tensor_copy(
                        xt_sb[:, ph * 2:(ph + 1) * 2, :], xt_ps[:])
                else:
                    nc.vector.tensor_copy(
                        xt_sb[:, ph * 2, :w], xt_ps[:, :w])
                    nc.vector.tensor_copy(
                        xt_sb[:, ph * 2 + 1, :w], xt_ps[:, w:2 * w])
            st[i] = {"x": x_bf, "xt": xt_sb}

        def stage_b(i):
            bb, j, q0, nq = macros[i]
            w = nq * 128
            if j == 0:
                batch[bb] = {"sums": statp.tile([M, n_eff[bb]], f32, tag="sums", name="sums")}
            xt_sb = st[i]["xt"]
            sc_ps = ps_sc.tile([M, S_MACRO], f32, tag="sc", name="sc")
            for dc in range(DC):
                nc.tensor.matmul(
                    sc_ps[:, :w],
                    lhsT=seedsT[:, dc * M:(dc + 1) * M],
                    rhs=xt_sb[:, dc, :w],
                    start=(dc == 0),
                    stop=(dc == DC - 1),
                )
            e_bf = ep.tile([M, S_MACRO], bf16, tag="e", name="e_bf")
            nc.scalar.activation(
                e_bf[:, :w], sc_ps[:, :w], mybir.ActivationFunctionType.Exp,
                accum_out=batch[bb]["sums"][:, j:j + 1],
            )
            st[i]["e"] = e_bf
            if j == n_eff[bb] - 1:
                # sums complete after this exp; compute recip early so the
                # batch-end finalize chain starts without it
                total = statp.tile([M, 1], f32, tag="tot", name="tot")
                nc.vector.reduce_sum(
                    total[:], batch[bb]["sums"][:], axis=mybir.AxisListType.X)
                recip = statp.tile([M, 1], f32, tag="rec", name="rec")
                nc.vector.reciprocal(recip[:], total[:])
                batch[bb]["recip"] = recip

        def stage_c1(i):
            # expT transposes + small DVE copy; runs while scores(i+1) stream
            bb, j, q0, nq = macros[i]
            if j == 0:
                batch[bb]["pl"] = ps_pl.tile([128, D], f32, tag="pl", name="pl")
            e_bf = st[i]["e"]
            et_ps = ps_et.tile([128, T_SUB * M], bf16, tag="et", name="et")
            for t in range(nq):
                nc.tensor.transpose(
                    et_ps[:, t * M:(t + 1) * M],
                    e_bf[:, t * 128:(t + 1) * 128],
                    ident[:M, :M],
                )
            et_sb = etp.tile([128, T_SUB * M], bf16, tag="et", name="et")
            nc.vector.tensor_copy(et_sb[:, :nq * M], et_ps[:, :nq * M])
            st[i]["et"] = et_sb

        def stage_c2(i):
            bb, j, q0, nq = macros[i]
            x_bf = st[i]["x"]
            et_sb = st[i]["et"]
            pool_ps = batch[bb]["pl"]
            # 2-way column-tiled: two col-groups run concurrently on the
            # PE (2 serial matmuls each; 1 each for the 2-q minis); only
            # 2 psum partials per batch, so the batch-end finalize is one
            # ACT + one DVE op.
            for t in range(nq):
                g = t // 2 if nq == T_SUB else t
                nc.tensor.matmul(
                    pool_ps[64 * g:64 * g + M, :],
                    lhsT=et_sb[:, t * M:(t + 1) * M],
                    rhs=x_bf[:, t, :],
                    start=(j == 0 and t % 2 == 0),
                    stop=(j == n_eff[bb] - 1 and (nq != T_SUB or t % 2 == 1)),
                    tile_position=(0, 64 * g),
                    skip_group_check=True,
                )
            del st[i]
            del chunk_tiles[i]
            if j == n_eff[bb] - 1:
                recip = batch[bb]["recip"]
                # o = (partial0 + partial1) * recip, split ACT/DVE
                o_sb = outp.tile([M, D], f32, tag="o", name="o_sb")
                nc.scalar.activation(
                    o_sb[:], pool_ps[0:M, :],
                    mybir.ActivationFunctionType.Copy, scale=recip[:],
                )
                nc.vector.scalar_tensor_tensor(
                    o_sb[:], pool_ps[64:64 + M, :], recip[:],
                    o_sb[:], op0=mybir.AluOpType.mult,
                    op1=mybir.AluOpType.add,
                )
                nc.scalar.dma_start(out=out_ap[bb], in_=o_sb[:])
                del batch[bb]

        for i in range(NM + 3):
            if i < NM and i + PREFETCH < NM:
                stage_dma(i + PREFETCH)
            if 3 <= i <= NM + 2:
                stage_c1(i - 3)
            if 2 <= i <= NM + 1:
                stage_b(i - 2)
            if 3 <= i <= NM + 2:
                stage_c2(i - 3)
            if i < NM:
                stage_a(i)


def build_bass(b_loc, s):
    nc = bacc.Bacc(
        "TRN2", target_bir_lowering=False, debug=False, num_devices=N_CORES
    )
    x_d = nc.dram_tensor("x", [b_loc, s, D], f32, kind="ExternalInput")
    seeds_d = nc.dram_tensor("seeds", [M, D], f32, kind="ExternalInput")
    out_d = nc.dram_tensor("out", [b_loc, M, D], f32, kind="ExternalOutput")
    with tile.TileContext(nc) as tc:
        kernel_body(tc, out_d.ap(), x_d.ap(), seeds_d.ap(), b_loc, s)
    nc.compile()
    return nc


_cached = {}


def get_nc(b_loc, s):
    key = (b_loc, s)
    if key not in _cached:
        _cached[key] = build_bass(b_loc, s)
    return _cached[key]


def kernel(x, seeds, trace=False):
    assert x.shape == (B, S, D) and seeds.shape == (M, D)
    x = np.asarray(x, dtype=np.float32)
    seeds = np.asarray(seeds, dtype=np.float32)
    b_loc = B // N_CORES
    nc = get_nc(b_loc, S)
    in_maps = [
        {
            "x": np.ascontiguousarray(x[i * b_loc:(i + 1) * b_loc]),
            "seeds": seeds,
        }
        for i in range(N_CORES)
    ]
    res = run_bass_kernel_spmd(
        nc, in_maps, core_ids=list(range(N_CORES)), trace=trace
    )
    out = np.concatenate([r["out"] for r in res.results], axis=0)
    if trace:
        kernel.last_result = res
    return out.astype(np.float32)


kernel.last_result = None


# revision 27
# speedup vs baseline: 1.1136x; 1.1136x over previous
"""Attention-pooling kernel for TRN2 (8 NeuronCores, data-parallel over batch).

Computes, per batch b:
    scores = seeds @ x[b].T          # [M, S]
    weights = softmax(scores, -1)
    out[b] = weights @ x[b]          # [M, D]

Sharding: batch B=32 split 4-per-core across 8 cores; seeds replicated.

Per-core pipeline (all bf16 on-chip, f32 PSUM accumulation):
  - SWDGE cast-DMA loads x HBM f32 -> SBUF bf16 in uniform 1 MB chunks
    (one per 512-row macro; the final two macros split into 0.5 MB
    minis). The x stream is the critical path (~67 MB at ~340-390 GB/s
    effective HBM rate), so the chunk DMAs are the first instructions
    on the gpsimd queue and the seeds load rides the HWDGE (scalar)
    queue instead. Per-macro completion granularity keeps PE data waits
    small enough that PE_HAM never half-clocks the array.
  - PE transposes x 128x128 blocks (bf16) -> psum -> DVE copies to SBUF
    as x^T chunks.
  - scores: 4 accumulating matmuls lhsT=seedsT chunk [128,16], rhs=xT,
    into a double-buffered scores psum so scores(i) never waits for
    exp(i-1) to drain it.
  - exp on ACT straight out of PSUM, with fused accum_out row-sums
    (no max subtraction: scores = seeds.x are bounded ~|8|, exp is safe
    in f32).
  - PE transposes exp [16,128] -> expT [128,16]; pooled matmuls are
    2-way column-tiled (concurrent PE col-groups), partials accumulate
    in psum rows {0,64}+[0,16) over the whole batch.
  - batch end: o = (partial0 + partial1) * recip(sum), split across ACT
    (activation Copy with scale) and DVE (scalar_tensor_tensor), then
    DMA out f32.
  - Stages are software-pipelined (C(i-3), B(i-2), A(i)) so every
    cross-engine round trip has a full macro of slack and the PE never
    stalls mid-macro.
"""

from contextlib import ExitStack

import numpy as np

import concourse.mybir as mybir
import concourse.tile as tile
from concourse import bacc
from concourse.bass_utils import run_bass_kernel_spmd
from concourse.masks import make_identity

N_CORES = 8
B, S, D, M = 32, 8192, 512, 16
S_MACRO = 512           # s rows per macro-tile
T_SUB = S_MACRO // 128  # 128-row subtiles per macro-tile
DC = D // 128            # 128-col d chunks
XP_BUFS = 12             # 1 MB chunk tiles in flight (4 KB/partition each)

f32 = mybir.dt.float32
bf16 = mybir.dt.bfloat16


def batch_macros(bb, b_loc, n_q):
    """(q0, nq) macro extents for one batch, in 128-row q units.

    One DMA chunk == one macro. Uniform 4-q (1 MB) macros: the SWDGE
    stream pipelines packets continuously regardless of chunk size, and
    per-macro completion granularity keeps the PE's data waits tiny
    (~0.3 us) so the PE_HAM activity monitor never sees an idle window
    and never half-clocks the PE (4 MB chunks caused ~2 us waits -> K=4
    windows -> ~2.5 us of extra PE time after every chunk boundary).
    The very last two macros are split into four 2-q (0.5 MB) minis so
    the post-stream dependency chain (transpose -> copy -> scores ->
    exp -> eT -> pooled) on the final macro is half as long.
    """
    if bb < b_loc - 1:
        return [(4 * k, 4) for k in range(n_q // 4)]
    n4 = n_q // 4 - 2
    return [(4 * k, 4) for k in range(n4)] + \
           [(4 * n4 + 2 * k, 2) for k in range(4)]


def kernel_body(tc, out_ap, x_ap, seeds_ap, b_loc, s):
    nc = tc.nc
    n_mac = s // S_MACRO
    with ExitStack() as ctx:
        const = ctx.enter_context(tc.tile_pool(name="const", bufs=1))
        xp = ctx.enter_context(tc.tile_pool(name="xp", bufs=XP_BUFS))
        xtp = ctx.enter_context(tc.tile_pool(name="xtp", bufs=4))
        ep = ctx.enter_context(tc.tile_pool(name="ep", bufs=4))
        etp = ctx.enter_context(tc.tile_pool(name="etp", bufs=4))
        statp = ctx.enter_context(tc.tile_pool(name="statp", bufs=4))
        outp = ctx.enter_context(tc.tile_pool(name="outp", bufs=2))
        ps_xt = ctx.enter_context(tc.tile_pool(name="ps_xt", bufs=4, space="PSUM"))
        # sc bufs=2: scores(i) must not wait for exp(i-1) to drain the
        # scores psum — that PE<->ACT ping-pong serializes the post-stream
        # drain. pl bufs=1 is safe: the next batch's first pooled matmul
        # lands ~2.6 us after the previous batch's finalize reads.
        ps_sc = ctx.enter_context(tc.tile_pool(name="ps_sc", bufs=2, space="PSUM"))
        ps_et = ctx.enter_context(tc.tile_pool(name="ps_et", bufs=1, space="PSUM"))
        ps_pl = ctx.enter_context(tc.tile_pool(name="ps_pl", bufs=1, space="PSUM"))

        # x view per batch: [b, p, q, d] with s = p*(s/128) + q. Partition p
        # holds s/128=64 consecutive s rows (128 KB contiguous HBM), so any
        # q-slice is a contiguous per-partition read. The s-order is a fixed
        # permutation; softmax is permutation-invariant and scores/exp/pooled
        # all use the same block mapping, so it cancels.
        x_rb = x_ap.rearrange("b (p q) d -> b p q d", p=128)

        # macro schedule: (bb, j within batch, q0, nq); chunk == macro
        n_q = s // 128
        macros = []
        n_eff = {}
        for bb in range(b_loc):
            bm = batch_macros(bb, b_loc, n_q)
            n_eff[bb] = len(bm)
            for j, (q0, nq) in enumerate(bm):
                macros.append((bb, j, q0, nq))
        NM = len(macros)

        chunk_tiles = {}

        def stage_dma(i):
            bb, j, q0, nq = macros[i]
            x_bf = xp.tile([128, T_SUB, D], bf16, tag="x", name="x_bf")
            nc.gpsimd.dma_start(
                out=x_bf[:, :nq, :],
                in_=x_rb[bb, :, q0:q0 + nq, :],
            )
            chunk_tiles[i] = x_bf

        # x chunk DMAs are the first gpsimd-queue work so SDMA starts
        # streaming immediately; identity (also gpsimd: memset +
        # affine_select) slots in after two emissions, well before the
        # first transposes need it.
        PREFETCH = XP_BUFS - 1
        stage_dma(0)
        stage_dma(1)

        ident = const.tile([128, 128], bf16)
        make_identity(nc, ident)

        # seeds on the HWDGE (scalar) queue, f32; DVE casts to bf16.
        seeds_f = const.tile([M, D], f32)
        nc.scalar.dma_start(out=seeds_f[:], in_=seeds_ap)
        seeds_bf = const.tile([M, D], bf16)
        nc.vector.tensor_copy(seeds_bf[:], seeds_f[:])

        for c in range(2, min(PREFETCH, NM)):
            stage_dma(c)

        # seeds -> seedsT [d, m] chunks, [128, DC*M] (dc-major)
        ps_st = ps_et.tile([128, DC * M], bf16, tag="et", name="et")
        for dc in range(DC):
            nc.tensor.transpose(
                ps_st[:, dc * M:(dc + 1) * M],
                seeds_bf[:, dc * 128:(dc + 1) * 128],
                ident[:M, :M],
            )
        seedsT = const.tile([128, DC * M], bf16)
        nc.vector.tensor_copy(seedsT[:], ps_st[:])

        # Software-pipelined across all (batch, macro) pairs:
        #   DMA: one cast-load per chunk, PREFETCH chunks ahead
        #   stage A(i): PE x-transposes, DVE psum->sbuf copies
        #   stage B(i): scores matmuls, ACT exp (+row-sum)
        #   stage C(i): PE exp-transposes, DVE copy, pooled matmuls, finalize
        # B runs 2 macros behind A and C 3 behind, so every cross-engine
        # round trip (PE->DVE xt copy->scores; ACT exp->eT) has a full
        # macro of slack and the PE never stalls mid-macro.
        st = {}  # per-macro live tiles
        batch = {}  # per-batch state: sums tile, pool psum

        def stage_a(i):
            bb, j, q0, nq = macros[i]
            x_bf = chunk_tiles[i]
            w = nq * 128
            xt_sb = xtp.tile([128, DC, S_MACRO], bf16, tag="xt", name="xt")
            for ph in range(DC // 2):  # 2 dc chunks per psum bank
                xt_ps = ps_xt.tile([128, 2 * S_MACRO], bf16, tag="xt", name="xt")
                for dch in range(2):
                    dc = ph * 2 + dch
                    for t in range(nq):
                        nc.tensor.transpose(
                            xt_ps[:, dch * w + t * 128:dch * w + (t + 1) * 128],
                            x_bf[:, t, dc * 128:(dc + 1) * 128],
                            ident[:],
                        )
                if nq == T_SUB:
                    nc.vector.tensor_copy(
                        xt_sb[:, ph * 2:(ph + 1) * 2, :], xt_ps[:])
                else:
                    nc.vector.tensor_copy(
                        xt_sb[:, ph * 2, :w], xt_ps[:, :w])
                    nc.vector.tensor_copy(
                        xt_sb[:, ph * 2 + 1, :w], xt_ps[:, w:2 * w])
            st[i] = {"x": x_bf, "xt": xt_sb}

        def stage_b(i):
            bb, j, q0, nq = macros[i]
            w = nq * 128
            if j == 0:
                batch[bb] = {"sums": statp.tile([M, n_eff[bb]], f32, tag="sums", name="sums")}
            xt_sb = st[i]["xt"]
            sc_ps = ps_sc.tile([M, S_MACRO], f32, tag="sc", name="sc")
            for dc in range(DC):
                nc.tensor.matmul(
                    sc_ps[:, :w],
                    lhsT=seedsT[:, dc * M:(dc + 1) * M],
                    rhs=xt_sb[:, dc, :w],
                    start=(dc == 0),
                    stop=(dc == DC - 1),
                )
            e_bf = ep.tile([M, S_MACRO], bf16, tag="e", name="e_bf")
            nc.scalar.activation(
                e_bf[:, :w], sc_ps[:, :w], mybir.ActivationFunctionType.Exp,
                accum_out=batch[bb]["sums"][:, j:j + 1],
            )
            st[i]["e"] = e_bf
            if j == n_eff[bb] - 1:
                # sums complete after this exp; compute recip early so the
                # batch-end finalize chain starts without it
                total = statp.tile([M, 1], f32, tag="tot", name="tot")
                nc.vector.reduce_sum(
                    total[:], batch[bb]["sums"][:], axis=mybir.AxisListType.X)
                recip = statp.tile([M, 1], f32, tag="rec", name="rec")
                nc.vector.reciprocal(recip[:], total[:])
                batch[bb]["recip"] = recip

        def stage_c1(i):
            # expT transposes + small DVE copy; runs while scores(i+1) stream
            bb, j, q0, nq = macros[i]
            if j == 0:
                batch[bb]["pl"] = ps_pl.tile([128, D], f32, tag="pl", name="pl")
            e_bf = st[i]["e"]
            et_ps = ps_et.tile([128, T_SUB * M], bf16, tag="et", name="et")
            for t in range(nq):
                nc.tensor.transpose(
                    et_ps[:, t * M:(t + 1) * M],
                    e_bf[:, t * 128:(t + 1) * 128],
                    ident[:M, :M],
                )
            et_sb = etp.tile([128, T_SUB * M], bf16, tag="et", name="et")
            nc.vector.tensor_copy(et_sb[:, :nq * M], et_ps[:, :nq * M])
            st[i]["et"] = et_sb

        def stage_c2(i):
            bb, j, q0, nq = macros[i]
            x_bf = st[i]["x"]
            et_sb = st[i]["et"]
            pool_ps = batch[bb]["pl"]
            # 2-way column-tiled: two col-groups run concurrently on the
            # PE (2 serial matmuls each; 1 each for the 2-q minis); only
            # 2 psum partials per batch, so the batch-end finalize is one
            # ACT + one DVE op.
            for t in range(nq):
                g = t // 2 if nq == T_SUB else t
                nc.tensor.matmul(
                    pool_ps[64 * g:64 * g + M, :],
                    lhsT=et_sb[:, t * M:(t + 1) * M],
                    rhs=x_bf[:, t, :],
                    start=(j == 0 and t % 2 == 0),
                    stop=(j == n_eff[bb] - 1 and (nq != T_SUB or t % 2 == 1)),
                    tile_position=(0, 64 * g),
                    skip_group_check=True,
                )
            del st[i]
            del chunk_tiles[i]
            if j == n_eff[bb] - 1:
                recip = batch[bb]["recip"]
                # o = (partial0 + partial1) * recip, split ACT/DVE
                o_sb = outp.tile([M, D], f32, tag="o", name="o_sb")
                nc.scalar.activation(
                    o_sb[:], pool_ps[0:M, :],
                    mybir.ActivationFunctionType.Copy, scale=recip[:],
                )
                nc.vector.scalar_tensor_tensor(
                    o_sb[:], pool_ps[64:64 + M, :], recip[:],
                    o_sb[:], op0=mybir.AluOpType.mult,
                    op1=mybir.AluOpType.add,
                )
                nc.scalar.dma_start(out=out_ap[bb], in_=o_sb[:])
                del batch[bb]

        for i in range(NM + 3):
            if i < NM and i + PREFETCH < NM:
                stage_dma(i + PREFETCH)
            if 3 <= i <= NM + 2:
                stage_c1(i - 3)
            if 2 <= i <= NM + 1:
                stage_b(i - 2)
            if 3 <= i <= NM + 2:
                stage_c2(i - 3)
            if i < NM:
                stage_a(i)


def build_bass(b_loc, s):
    nc = bacc.Bacc(
        "TRN2", target_bir_lowering=False, debug=False, num_devices=N_CORES
    )
    x_d = nc.dram_tensor("x", [b_loc, s, D], f32, kind="ExternalInput")
    seeds_d = nc.dram_tensor("seeds", [M, D], f32, kind="ExternalInput")
    out_d = nc.dram_tensor("out", [b_loc, M, D], f32, kind="ExternalOutput")
    with tile.TileContext(nc) as tc:
        kernel_body(tc, out_d.ap(), x_d.ap(), seeds_d.ap(), b_loc, s)
    nc.compile()
    return nc


_cached = {}


def get_nc(b_loc, s):
    key = (b_loc, s)
    if key not in _cached:
        _cached[key] = build_bass(b_loc, s)
    return _cached[key]


def kernel(x, seeds, trace=False):
    assert x.shape == (B, S, D) and seeds.shape == (M, D)
    x = np.asarray(x, dtype=np.float32)
    seeds = np.asarray(seeds, dtype=np.float32)
    b_loc = B // N_CORES
    nc = get_nc(b_loc, S)
    in_maps = [
        {
            "x": np.ascontiguousarray(x[i * b_loc:(i + 1) * b_loc]),
            "seeds": seeds,
        }
        for i in range(N_CORES)
    ]
    res = run_bass_kernel_spmd(
        nc, in_maps, core_ids=list(range(N_CORES)), trace=trace
    )
    out = np.concatenate([r["out"] for r in res.results], axis=0)
    if trace:
        kernel.last_result = res
    return out.astype(np.float32)


kernel.last_result = None


# revision 30
# speedup vs baseline: 1.1278x; 1.0128x over previous
"""Attention-pooling kernel for TRN2 (8 NeuronCores, data-parallel over batch).

Computes, per batch b:
    scores = seeds @ x[b].T          # [M, S]
    weights = softmax(scores, -1)
    out[b] = weights @ x[b]          # [M, D]

Sharding: batch B=32 split 4-per-core across 8 cores; seeds replicated.

Per-core pipeline (all bf16 on-chip, f32 PSUM accumulation):
  - SWDGE cast-DMA loads x HBM f32 -> SBUF bf16 in uniform 1 MB chunks
    (one per 512-row macro; the final two macros split into 0.5 MB
    minis). The x stream is the critical path (~67 MB at ~340-390 GB/s
    effective HBM rate), so the chunk DMAs are the first instructions
    on the gpsimd queue and the seeds load rides the HWDGE (scalar)
    queue instead. Per-macro completion granularity keeps PE data waits
    small enough that PE_HAM never half-clocks the array.
  - PE transposes x 128x128 blocks (bf16) -> psum -> DVE copies to SBUF
    as x^T chunks.
  - scores: 4 accumulating matmuls lhsT=seedsT chunk [128,16], rhs=xT,
    into a double-buffered scores psum so scores(i) never waits for
    exp(i-1) to drain it.
  - exp on ACT straight out of PSUM, with fused accum_out row-sums
    (no max subtraction: scores = seeds.x are bounded ~|8|, exp is safe
    in f32).
  - PE transposes exp [16,128] -> expT [128,16]; pooled matmuls are
    2-way column-tiled (concurrent PE col-groups), partials accumulate
    in psum rows {0,64}+[0,16) over the whole batch.
  - batch end: o = (partial0 + partial1) * recip(sum), split across ACT
    (activation Copy with scale) and DVE (scalar_tensor_tensor), then
    DMA out f32.
  - Stages are software-pipelined (C(i-3), B(i-2), A(i)) so every
    cross-engine round trip has a full macro of slack and the PE never
    stalls mid-macro.
"""

from contextlib import ExitStack

import numpy as np

import concourse.mybir as mybir
import concourse.tile as tile
from concourse import bacc
from concourse.bass_utils import run_bass_kernel_spmd
from concourse.masks import make_identity

N_CORES = 8
B, S, D, M = 32, 8192, 512, 16
S_MACRO = 512           # s rows per macro-tile
T_SUB = S_MACRO // 128  # 128-row subtiles per macro-tile
DC = D // 128            # 128-col d chunks
XP_BUFS = 12             # 1 MB chunk tiles in flight (4 KB/partition each)

f32 = mybir.dt.float32
bf16 = mybir.dt.bfloat16


def batch_macros(bb, b_loc, n_q):
    """(q0, nq) macro extents for one batch, in 128-row q units.

    One DMA chunk == one macro. Uniform 4-q (1 MB) macros: the SWDGE
    stream pipelines packets continuously regardless of chunk size, and
    per-macro completion granularity keeps the PE's data waits tiny
    (~0.3 us) so the PE_HAM activity monitor never sees an idle window
    and never half-clocks the PE (4 MB chunks caused ~2 us waits -> K=4
    windows -> ~2.5 us of extra PE time after every chunk boundary).
    The very last two macros are split into four 2-q (0.5 MB) minis so
    the post-stream dependency chain (transpose -> copy -> scores ->
    exp -> eT -> pooled) on the final macro is half as long.
    """
    if bb < b_loc - 1:
        return [(4 * k, 4) for k in range(n_q // 4)]
    n4 = n_q // 4 - 2
    return [(4 * k, 4) for k in range(n4)] + \
           [(4 * n4 + 2 * k, 2) for k in range(4)]


def kernel_body(tc, out_ap, x_ap, seeds_ap, b_loc, s):
    nc = tc.nc
    n_mac = s // S_MACRO
    with ExitStack() as ctx:
        const = ctx.enter_context(tc.tile_pool(name="const", bufs=1))
        xp = ctx.enter_context(tc.tile_pool(name="xp", bufs=XP_BUFS))
        xtp = ctx.enter_context(tc.tile_pool(name="xtp", bufs=4))
        ep = ctx.enter_context(tc.tile_pool(name="ep", bufs=4))
        etp = ctx.enter_context(tc.tile_pool(name="etp", bufs=4))
        statp = ctx.enter_context(tc.tile_pool(name="statp", bufs=4))
        outp = ctx.enter_context(tc.tile_pool(name="outp", bufs=2))
        pre01 = ctx.enter_context(tc.tile_pool(name="pre01", bufs=1))
        ps_xt = ctx.enter_context(tc.tile_pool(name="ps_xt", bufs=4, space="PSUM"))
        # sc bufs=2: scores(i) must not wait for exp(i-1) to drain the
        # scores psum — that PE<->ACT ping-pong serializes the post-stream
        # drain. pl bufs=1 is safe: the next batch's first pooled matmul
        # lands ~2.6 us after the previous batch's finalize reads.
        ps_sc = ctx.enter_context(tc.tile_pool(name="ps_sc", bufs=2, space="PSUM"))
        ps_et = ctx.enter_context(tc.tile_pool(name="ps_et", bufs=1, space="PSUM"))
        ps_pl = ctx.enter_context(tc.tile_pool(name="ps_pl", bufs=1, space="PSUM"))

        # x view per batch: [b, p, q, d] with s = p*(s/128) + q. Partition p
        # holds s/128=64 consecutive s rows (128 KB contiguous HBM), so any
        # q-slice is a contiguous per-partition read. The s-order is a fixed
        # permutation; softmax is permutation-invariant and scores/exp/pooled
        # all use the same block mapping, so it cancels.
        x_rb = x_ap.rearrange("b (p q) d -> b p q d", p=128)

        # macro schedule: (bb, j within batch, q0, nq); chunk == macro
        n_q = s // 128
        macros = []
        n_eff = {}
        for bb in range(b_loc):
            bm = batch_macros(bb, b_loc, n_q)
            n_eff[bb] = len(bm)
            for j, (q0, nq) in enumerate(bm):
                macros.append((bb, j, q0, nq))
        NM = len(macros)

        chunk_tiles = {}

        def stage_dma(i):
            bb, j, q0, nq = macros[i]
            x_bf = xp.tile([128, T_SUB, D], bf16, tag="x", name="x_bf")
            nc.gpsimd.dma_start(
                out=x_bf[:, :nq, :],
                in_=x_rb[bb, :, q0:q0 + nq, :],
            )
            chunk_tiles[i] = x_bf

        # x chunk DMAs are the first gpsimd-queue work so SDMA starts
        # streaming immediately; identity (also gpsimd: memset +
        # affine_select) slots in after two emissions, well before the
        # first transposes need it.
        PREFETCH = XP_BUFS - 1
        stage_dma(0)
        stage_dma(1)

        ident = const.tile([128, 128], bf16)
        make_identity(nc, ident)

        # seeds on the HWDGE (scalar) queue, f32; DVE casts to bf16.
        seeds_f = const.tile([M, D], f32)
        nc.scalar.dma_start(out=seeds_f[:], in_=seeds_ap)
        seeds_bf = const.tile([M, D], bf16)
        nc.vector.tensor_copy(seeds_bf[:], seeds_f[:])

        for c in range(2, min(PREFETCH, NM)):
            stage_dma(c)

        # seeds -> seedsT [d, m] chunks, [128, DC*M] (dc-major)
        ps_st = ps_et.tile([128, DC * M], bf16, tag="et", name="et")
        for dc in range(DC):
            nc.tensor.transpose(
                ps_st[:, dc * M:(dc + 1) * M],
                seeds_bf[:, dc * 128:(dc + 1) * 128],
                ident[:M, :M],
            )
        seedsT = const.tile([128, DC * M], bf16)
        nc.vector.tensor_copy(seedsT[:], ps_st[:])

        # Software-pipelined across all (batch, macro) pairs:
        #   DMA: one cast-load per chunk, PREFETCH chunks ahead
        #   stage A(i): PE x-transposes, DVE psum->sbuf copies
        #   stage B(i): scores matmuls, ACT exp (+row-sum)
        #   stage C(i): PE exp-transposes, DVE copy, pooled matmuls, finalize
        # B runs 2 macros behind A and C 3 behind, so every cross-engine
        # round trip (PE->DVE xt copy->scores; ACT exp->eT) has a full
        # macro of slack and the PE never stalls mid-macro.
        st = {}  # per-macro live tiles
        batch = {}  # per-batch state: sums tile, pool psum

        def stage_a(i):
            bb, j, q0, nq = macros[i]
            x_bf = chunk_tiles[i]
            w = nq * 128
            xt_sb = xtp.tile([128, DC, S_MACRO], bf16, tag="xt", name="xt")
            for ph in range(DC // 2):  # 2 dc chunks per psum bank
                xt_ps = ps_xt.tile([128, 2 * S_MACRO], bf16, tag="xt", name="xt")
                for dch in range(2):
                    dc = ph * 2 + dch
                    for t in range(nq):
                        nc.tensor.transpose(
                            xt_ps[:, dch * w + t * 128:dch * w + (t + 1) * 128],
                            x_bf[:, t, dc * 128:(dc + 1) * 128],
                            ident[:],
                        )
                if nq == T_SUB:
                    nc.vector.tensor_copy(
                        xt_sb[:, ph * 2:(ph + 1) * 2, :], xt_ps[:])
                else:
                    nc.vector.tensor_copy(
                        xt_sb[:, ph * 2, :w], xt_ps[:, :w])
                    nc.vector.tensor_copy(
                        xt_sb[:, ph * 2 + 1, :w], xt_ps[:, w:2 * w])
            st[i] = {"x": x_bf, "xt": xt_sb}

        def stage_b(i):
            bb, j, q0, nq = macros[i]
            w = nq * 128
            if j == 0:
                batch[bb] = {"sums": statp.tile([M, n_eff[bb]], f32, tag="sums", name="sums")}
            xt_sb = st[i]["xt"]
            sc_ps = ps_sc.tile([M, S_MACRO], f32, tag="sc", name="sc")
            for dc in range(DC):
                nc.tensor.matmul(
                    sc_ps[:, :w],
                    lhsT=seedsT[:, dc * M:(dc + 1) * M],
                    rhs=xt_sb[:, dc, :w],
                    start=(dc == 0),
                    stop=(dc == DC - 1),
                )
            e_bf = ep.tile([M, S_MACRO], bf16, tag="e", name="e_bf")
            nc.scalar.activation(
                e_bf[:, :w], sc_ps[:, :w], mybir.ActivationFunctionType.Exp,
                accum_out=batch[bb]["sums"][:, j:j + 1],
            )
            st[i]["e"] = e_bf
            if j == n_eff[bb] - 1:
                # sums complete after this exp; compute recip early so the
                # batch-end finalize chain starts without it
                total = statp.tile([M, 1], f32, tag="tot", name="tot")
                nc.vector.reduce_sum(
                    total[:], batch[bb]["sums"][:], axis=mybir.AxisListType.X)
                recip = statp.tile([M, 1], f32, tag="rec", name="rec")
                nc.vector.reciprocal(recip[:], total[:])
                batch[bb]["recip"] = recip

        def stage_c1(i):
            # expT transposes + small DVE copy; runs while scores(i+1) stream
            bb, j, q0, nq = macros[i]
            if j == 0:
                batch[bb]["pl"] = ps_pl.tile([128, D], f32, tag="pl", name="pl")
            e_bf = st[i]["e"]
            et_ps = ps_et.tile([128, T_SUB * M], bf16, tag="et", name="et")
            for t in range(nq):
                nc.tensor.transpose(
                    et_ps[:, t * M:(t + 1) * M],
                    e_bf[:, t * 128:(t + 1) * 128],
                    ident[:M, :M],
                )
            et_sb = etp.tile([128, T_SUB * M], bf16, tag="et", name="et")
            nc.vector.tensor_copy(et_sb[:, :nq * M], et_ps[:, :nq * M])
            st[i]["et"] = et_sb

        def stage_c2(i):
            bb, j, q0, nq = macros[i]
            x_bf = st[i]["x"]
            et_sb = st[i]["et"]
            pool_ps = batch[bb]["pl"]
            last4 = n_eff[bb] - 4
            # 4-way column-tiled: each subtile runs in its own 32-col
            # group of the PE array with its own XBUS stream; the 4
            # streams run concurrently (~0.2 us wall/macro vs 0.4 for
            # 2-way). Minis route their 2 subtiles to alternating group
            # pairs so groups {0,1} stop one mini early and pre-reduce
            # while the last mini streams: only 2 DVE ops + 1 ACT op of
            # finalize remain exposed at batch end.
            for t in range(nq):
                if nq == T_SUB:
                    g = t
                    start = (j == 0)
                    stop = (j == n_eff[bb] - 1)
                else:
                    j_rel = j - last4
                    g = 2 * (j_rel % 2) + t
                    start = False
                    stop = (j_rel >= 2)
                nc.tensor.matmul(
                    pool_ps[32 * g:32 * g + M, :],
                    lhsT=et_sb[:, t * M:(t + 1) * M],
                    rhs=x_bf[:, t, :],
                    start=start,
                    stop=stop,
                    tile_position=(0, 32 * g),
                    skip_group_check=True,
                )
            del st[i]
            del chunk_tiles[i]
            if j == n_eff[bb] - 1:
                recip = batch[bb]["recip"]
                o_sb = outp.tile([M, D], f32, tag="o", name="o_sb")
                nc.scalar.activation(
                    o_sb[:], pool_ps[0:M, :],
                    mybir.ActivationFunctionType.Copy, scale=recip[:],
                )
                for g in (1, 2, 3):
                    nc.vector.scalar_tensor_tensor(
                        o_sb[:], pool_ps[32 * g:32 * g + M, :], recip[:],
                        o_sb[:], op0=mybir.AluOpType.mult,
                        op1=mybir.AluOpType.add,
                    )
                nc.scalar.dma_start(out=out_ap[bb], in_=o_sb[:])
                del batch[bb]

        for i in range(NM + 3):
            if i < NM and i + PREFETCH < NM:
                stage_dma(i + PREFETCH)
            if 3 <= i <= NM + 2:
                stage_c1(i - 3)
            if 2 <= i <= NM + 1:
                stage_b(i - 2)
            if 3 <= i <= NM + 2:
                stage_c2(i - 3)
            if i < NM:
                stage_a(i)


def build_bass(b_loc, s):
    nc = bacc.Bacc(
        "TRN2", target_bir_lowering=False, debug=False, num_devices=N_CORES
    )
    x_d = nc.dram_tensor("x", [b_loc, s, D], f32, kind="ExternalInput")
    seeds_d = nc.dram_tensor("seeds", [M, D], f32, kind="ExternalInput")
    out_d = nc.dram_tensor("out", [b_loc, M, D], f32, kind="ExternalOutput")
    with tile.TileContext(nc) as tc:
        kernel_body(tc, out_d.ap(), x_d.ap(), seeds_d.ap(), b_loc, s)
    nc.compile()
    return nc


_cached = {}


def get_nc(b_loc, s):
    key = (b_loc, s)
    if key not in _cached:
        _cached[key] = build_bass(b_loc, s)
    return _cached[key]


def kernel(x, seeds, trace=False):
    assert x.shape == (B, S, D) and seeds.shape == (M, D)
    x = np.asarray(x, dtype=np.float32)
    seeds = np.asarray(seeds, dtype=np.float32)
    b_loc = B // N_CORES
    nc = get_nc(b_loc, S)
    in_maps = [
        {
            "x": np.ascontiguousarray(x[i * b_loc:(i + 1) * b_loc]),
            "seeds": seeds,
        }
        for i in range(N_CORES)
    ]
    res = run_bass_kernel_spmd(
        nc, in_maps, core_ids=list(range(N_CORES)), trace=trace
    )
    out = np.concatenate([r["out"] for r in res.results], axis=0)
    if trace:
        kernel.last_result = res
    return out.astype(np.float32)


kernel.last_result = None
